# revision 52
# baseline (speedup 1.0000x reference)
"""Trainium2 Bass kernel for Transformer-XL style MHSA (nn_MHSAModule).

Problem (hardcoded):
  B=4, T=1024, D=512, H=8, DK=64, L=2*T-1=2047, eps=1e-3
  out = x + (MHSA(LayerNorm(x), pos) @ Wo + bo)

Sharding: 8 cores = 4 batches x 2 head-groups (4 heads each).
Core c handles batch c//2, heads 4*(c%2) .. 4*(c%2)+3. Each core returns a
partial output [T, D] (its heads' contribution); the host sums the two
partials per batch and adds the residual x + bo.

Device-side layout notes:
  - Activations feature-major: xT/yT [D, T]; per-head qT/kT [DK, T].
  - 1/sqrt(DK) is folded into the q evacuations (ACT scale=0.125, biases
    pre-scaled on the host), so exp() runs with scale=1 and the positional
    band values are small enough for an fp8e4m3 DRAM bounce.
  - The rel_shift is a skewed (stride BAND-1) fp8 re-read of the bounced
    band; the shifted band is added to the content scores in PSUM via an
    fp8 identity matmul.
  - LayerNorm stats use an all-ones [128,128] stationary so the column
    sums land replicated across every partition: the mean/var/rsqrt chain
    runs on replicated tiles and the a/b rows never touch DRAM. The apply
    splits its chunks 3:1 between DVE and the otherwise-idle GPSIMD.
  - Dtypes by path: positional qp/pT/wp fp16 and posT fp8 (full PE rate,
    minimal DMA; the band is fp8-quantized downstream anyway); projections
    bf16 (y, Wq/Wk/Wv/Wo); content scores fp32r; LN stats fp32r-exact.
  - attn@V writes each head into its parity half of a pair-stacked
    oT [128, 2T] (PSUM base-partition-64 matmul writes), so the output
    projection contracts a full 128 lanes per head pair.
  - Emission is software-pipelined: band matmuls + bounce of head h+2 and
    the skewed re-reads of head h+1 are woven between the attention
    qbp-groups of head h, so the PE never waits on the bounce round trip.
    Each head gets its own DRAM bounce slot (no write-after-read reuse).
    Tiny warm-up activations at t=0 pull all ACT table loads into the
    initial DMA window.
"""
import numpy as np
from contextlib import ExitStack

import concourse.bass as bass
import concourse.bacc as bacc
import concourse.tile as tile
from concourse import mybir
from concourse import masks
from concourse.bass_utils import run_bass_kernel_spmd

F32 = mybir.dt.float32
F32R = mybir.dt.float32r
F16 = mybir.dt.float16
BF16 = mybir.dt.bfloat16
F8 = mybir.dt.float8e4
AF = mybir.ActivationFunctionType
OP = mybir.AluOpType

B, T, D, H, DK = 4, 1024, 512, 8, 64
L = 2 * T - 1
EPS = 1e-3
NH = 4          # heads per core
NP = 2          # head pairs per core
CH = D // 128   # 4 contraction chunks
QB = T // 128   # 8 q blocks
BAND = 1152     # positional band width per q block
BC = 384        # band psum chunk width (3 chunks of 384 = 1152)
PL = L + 2      # padded pT free size (2 zero pad cols)


def _build_program() -> bass.Bass:
    nc = bacc.Bacc("TRN2", target_bir_lowering=False, debug=False)

    # ---- DRAM I/O ----
    xT = nc.dram_tensor("xT", [D, T], F32R, kind="ExternalInput")
    posT = nc.dram_tensor("posT", [D, L], F8, kind="ExternalInput")
    wq = nc.dram_tensor("wq", [D, NH * DK], BF16, kind="ExternalInput")
    wk = nc.dram_tensor("wk", [D, NH * DK], BF16, kind="ExternalInput")
    wv = nc.dram_tensor("wv", [D, NH * DK], BF16, kind="ExternalInput")
    wp = nc.dram_tensor("wp", [D, NH * DK], F16, kind="ExternalInput")
    wo = nc.dram_tensor("wo", [2 * DK, NP * D], BF16, kind="ExternalInput")
    qc_bias = nc.dram_tensor("qc_bias", [128, NP], F32, kind="ExternalInput")
    qp_bias = nc.dram_tensor("qp_bias", [128, NP], F32, kind="ExternalInput")
    k_bias = nc.dram_tensor("k_bias", [128, NP], F32, kind="ExternalInput")
    v_bias = nc.dram_tensor("v_bias", [NH * DK], F32, kind="ExternalInput")
    out_d = nc.dram_tensor("out_partial", [T, D], F32, kind="ExternalOutput")

    # internal scratch
    bounce = nc.dram_tensor("bounce", [NH, QB, 128, BAND], F8)

    with tile.TileContext(nc) as tc, ExitStack() as ctx:
        sb = ctx.enter_context(tc.tile_pool(name="sb", bufs=1))
        sb2 = ctx.enter_context(tc.tile_pool(name="sb2", bufs=2))
        ps_sc = ctx.enter_context(tc.tile_pool(name="ps_sc", bufs=2, space="PSUM"))
        ps_b = ctx.enter_context(tc.tile_pool(name="ps_b", bufs=2, space="PSUM"))
        ps_m = ctx.enter_context(tc.tile_pool(name="ps_m", bufs=2, space="PSUM"))

        # ---- persistent SBUF ----
        xT_sb = sb.tile([128, CH * T], F32R)
        yT_sb = sb.tile([128, CH * T], BF16)
        posT_sb = sb.tile([128, CH * L + 2], F8)
        pT_sb = sb.tile([128, NP * PL], F16)
        qcT_sb = sb.tile([128, NP * T], F32R)
        qpT_sb = sb.tile([128, NP * T], F16)
        kT_sb = sb.tile([128, NP * T], F32R)
        v_sb = sb.tile([128, QB * NH * DK], F16)
        oT_sb = sb.tile([128, NP * T], BF16)
        wq_sb = sb.tile([128, CH * 256], BF16)
        wk_sb = sb.tile([128, CH * 256], BF16)
        wv_sb = sb.tile([128, CH * 256], BF16)
        wp_sb = sb.tile([128, CH * 256], F16)
        wo_sb = sb.tile([128, NP * D], BF16)
        qcb_sb = sb.tile([128, NP], F32)
        qpb_sb = sb.tile([128, NP], F32)
        kb_sb = sb.tile([128, NP], F32)
        vb_sb = sb.tile([128, 256], F32)
        arep = sb.tile([128, T], F32)
        brep = sb.tile([128, T], F32)
        ident16 = sb.tile([128, 128], F16)
        ident8 = sb.tile([128, 128], F8)
        ones128 = sb.tile([128, 128], F32)
        eps_col = sb.tile([128, 1], F32)

        masks.make_identity(nc, ident16[:])
        masks.make_identity(nc, ident8[:])
        nc.vector.memset(ones128[:], 1.0)
        nc.vector.memset(eps_col[:], EPS)

        # warm the ACT function tables during the initial load window so no
        # LoadActFuncSet lands on the stats/softmax critical chains
        warm = sb.tile([1, 1], F32)
        for wf in (AF.Exp, AF.Square, AF.Sqrt):
            nc.scalar.activation(warm[:], eps_col[0:1, :], wf)

        # ---- loads (one DMA per tensor; 3D APs fold the chunk dim) ----
        # tt0 half-chunks first: LN stats for the first token half can
        # start after 4 half loads instead of the full x transfer
        for tt in range(2):
            for c in range(CH):
                nc.sync.dma_start(
                    xT_sb[:, c * T + tt * 512: c * T + tt * 512 + 512],
                    xT[c * 128:(c + 1) * 128, tt * 512:(tt + 1) * 512])
        nc.sync.dma_start(
            posT_sb[:, :CH * L],
            bass.AP(posT[:].tensor, 0, [[L, 128], [128 * L, CH], [1, L]]))
        for w_sb, w_d in ((wp_sb, wp), (wq_sb, wq), (wk_sb, wk), (wv_sb, wv)):
            nc.sync.dma_start(
                w_sb[:],
                bass.AP(w_d[:].tensor, 0, [[256, 128], [128 * 256, CH], [1, 256]]))
        nc.sync.dma_start(wo_sb[:], wo[:])
        nc.sync.dma_start(qcb_sb[:], qc_bias[:])
        nc.sync.dma_start(qpb_sb[:], qp_bias[:])
        nc.sync.dma_start(kb_sb[:], k_bias[:])
        nc.sync.dma_start(
            vb_sb[:], bass.AP(v_bias[:].tensor, 0, [[0, 128], [1, 256]]))

        # ---- LayerNorm stats: an all-ones [128,128] stationary makes the
        # column-sum matmul land REPLICATED across all partitions, so the
        # a/b rows never need a DRAM replicate-bounce ----
        mu = sb.tile([128, 512], F32)
        var = sb.tile([128, 512], F32)
        std = sb.tile([128, 512], F32)
        for tt in range(2):
            ar = arep[:, tt * 512:(tt + 1) * 512]
            br = brep[:, tt * 512:(tt + 1) * 512]
            sums_ps = ps_m.tile([128, 512], F32, tag="misc")
            for c in range(CH):
                xt = xT_sb[:, c * T + tt * 512: c * T + tt * 512 + 512]
                nc.tensor.matmul(sums_ps[:], ones128[:].bitcast(F32R), xt,
                                 start=(c == 0), stop=(c == CH - 1))
            nc.vector.tensor_scalar_mul(mu[:], sums_ps[:], 1.0 / D)
            sumsq_ps = ps_m.tile([128, 512], F32, tag="misc")
            for c in range(CH):
                xsq = sb2.tile([128, 512], F32R, tag="xsq")
                xt = xT_sb[:, c * T + tt * 512: c * T + tt * 512 + 512]
                nc.scalar.activation(xsq[:], xt, AF.Square)
                nc.tensor.matmul(sumsq_ps[:], ones128[:].bitcast(F32R),
                                 xsq[:], start=(c == 0), stop=(c == CH - 1))
            nc.vector.tensor_tensor(var[:], mu[:], mu[:], op=OP.mult)
            nc.vector.scalar_tensor_tensor(var[:], sumsq_ps[:], 1.0 / D,
                                           var[:], op0=OP.mult,
                                           op1=OP.subtract)
            nc.scalar.activation(std[:], var[:], AF.Sqrt, bias=eps_col[:])
            nc.vector.reciprocal(ar, std[:])
            nc.vector.scalar_tensor_tensor(br, mu[:], -1.0, ar,
                                           op0=OP.mult, op1=OP.mult)

        # ---- p projection (needs only posT/wp; fills the lnrows gap) ----
        zrow = sb.tile([128, 2], F16)
        zrow8 = sb.tile([128, 2], F8)
        nc.vector.memset(zrow[:], 0.0)
        nc.vector.memset(zrow8[:], 0.0)
        nc.vector.tensor_copy(posT_sb[:, CH * L:], zrow8[:])
        for p in range(NP):
            for nt in range(4):
                pps = ps_m.tile([128, 512], F32, tag="misc")
                for c in range(CH):
                    nc.tensor.matmul(
                        pps[:],
                        wp_sb[:, c * 256 + p * 128: c * 256 + p * 128 + 128],
                        posT_sb[:, c * L + nt * 512: c * L + nt * 512 + 512],
                        start=(c == 0), stop=(c == CH - 1))
                nc.scalar.copy(
                    pT_sb[:, p * PL + nt * 512: p * PL + nt * 512 + 512],
                    pps[:])
        for p in range(NP):
            nc.vector.tensor_copy(pT_sb[:, p * PL + L: (p + 1) * PL], zrow[:])

        def emit_apply(tt):
            # yT = xT * a + b for one 512-token half; chunks split across
            # DVE and the otherwise-idle GPSIMD so the halves run in parallel
            for c in range(CH):
                eng = nc.vector if c < 3 else nc.gpsimd
                xs = xT_sb[:, c * T + tt * 512: c * T + tt * 512 + 512]
                ys = yT_sb[:, c * T + tt * 512: c * T + tt * 512 + 512]
                ar = arep[:, tt * 512:(tt + 1) * 512]
                br = brep[:, tt * 512:(tt + 1) * 512]
                eng.tensor_tensor(ys, xs, ar, op=OP.mult)
                eng.tensor_tensor(ys, ys, br, op=OP.add)

        def emit_qk(p, nt):
            o = p * T + nt * 512
            prj = ps_m.tile([128, 512], F32, tag="misc")
            for c in range(CH):
                nc.tensor.matmul(
                    prj[:],
                    wq_sb[:, c * 256 + p * 128: c * 256 + p * 128 + 128],
                    yT_sb[:, c * T + nt * 512: c * T + nt * 512 + 512],
                    start=(c == 0), stop=(c == CH - 1))
            nc.scalar.activation(qcT_sb[:, o:o + 512], prj[:], AF.Identity,
                                 bias=qcb_sb[:, p:p + 1], scale=0.125)
            nc.scalar.activation(qpT_sb[:, o:o + 512], prj[:], AF.Identity,
                                 bias=qpb_sb[:, p:p + 1], scale=0.125)
            prk = ps_m.tile([128, 512], F32, tag="misc")
            for c in range(CH):
                nc.tensor.matmul(
                    prk[:],
                    wk_sb[:, c * 256 + p * 128: c * 256 + p * 128 + 128],
                    yT_sb[:, c * T + nt * 512: c * T + nt * 512 + 512],
                    start=(c == 0), stop=(c == CH - 1))
            nc.scalar.activation(kT_sb[:, o:o + 512], prk[:], AF.Identity,
                                 bias=kb_sb[:, p:p + 1])

        def emit_v(t8):
            vps = ps_m.tile([128, 256], F32, tag="misc")
            for c in range(CH):
                nc.tensor.matmul(
                    vps[:],
                    yT_sb[:, c * T + t8 * 128: c * T + t8 * 128 + 128],
                    wv_sb[:, c * 256:(c + 1) * 256],
                    start=(c == 0), stop=(c == CH - 1))
            nc.vector.tensor_tensor(
                v_sb[:, t8 * 256:(t8 + 1) * 256], vps[:], vb_sb[:], op=OP.add)

        # ---- attention, software-pipelined over heads ----
        def head_views(h):
            p = h // 2
            off = (h % 2) * 64
            return (
                lambda lo, w: qpT_sb[off:off + 64, p * T + lo: p * T + lo + w],
                lambda lo, w: qcT_sb[off:off + 64, p * T + lo: p * T + lo + w],
                lambda lo, w: kT_sb[off:off + 64, p * T + lo: p * T + lo + w],
                lambda lo, w: pT_sb[off:off + 64, p * PL + lo: p * PL + lo + w],
            )

        shifted_tiles = {}

        def emit_band_qb(h, qb):
            """Band scores for one q block of head h: 3 matmul chunks,
            psum->fp8 evacuations (DVE, DVE, ACT), one bounce DMA."""
            qp_h, _, _, p_h = head_views(h)
            s0 = 897 - qb * 128
            stage = sb2.tile([128, BAND], F8, tag="stage", bufs=6)
            for bt in range(3):
                bps = ps_b.tile([128, BC], F32, tag="band")
                nc.tensor.matmul(bps[:], qp_h(qb * 128, 128),
                                 p_h(s0 + bt * BC, BC), start=True, stop=True)
                dst = stage[:, bt * BC:(bt + 1) * BC]
                if (bt < 2) if h >= 2 else (bt < 1):
                    nc.vector.tensor_copy(dst, bps[:])
                else:
                    nc.scalar.copy(dst, bps[:])
            nc.sync.dma_start(bounce[h, qb], stage[:])

        def alloc_shifted(h):
            shifted_tiles[h] = sb2.tile([128, QB * T], F8, tag="shift",
                                        bufs=3, name=f"shifted{h}")

        def emit_skew_read(h, part):
            # skewed re-read of bounce q-blocks [2*part, 2*part+1]; issued
            # right after those two bounce writes so the SP queue never
            # blocks long on this DMA's dependencies
            shifted = shifted_tiles[h]
            src = bass.AP(bounce[:].tensor,
                          h * (QB * 128 * BAND)
                          + 2 * part * (128 * BAND) + 127,
                          [[BAND - 1, 128], [128 * BAND, 2], [1, T]])
            nc.sync.dma_start(shifted[:, 2 * part * T: (2 * part + 2) * T], src)

        def emit_attn_qbp(h, qbp):
            """Scores + softmax + PE transpose + attn@V for 2 q blocks."""
            qp_h, qc_h, k_h, p_h = head_views(h)
            shifted = shifted_tiles[h]
            E_sb = sb2.tile([128, 2 * T], F16, tag="E", bufs=4)
            den = sb2.tile([128, 2], F32, tag="den", bufs=4)
            rec = sb2.tile([128, 2], F32, tag="rec", bufs=4)
            for qi in range(2):
                qb = qbp * 2 + qi
                sps = ps_sc.tile([128, T], F32, tag="scores")
                for nt in range(2):
                    nc.tensor.matmul(
                        sps[:, nt * 512: nt * 512 + 512],
                        qc_h(qb * 128, 128), k_h(nt * 512, 512),
                        start=True, stop=False)
                    if qb == 0 and nt == 1:
                        # scores[0, 1023] += (q+pos_bias)[1] . p[0]
                        nc.tensor.matmul(
                            sps[0:1, 1023:1024],
                            qp_h(1, 1), p_h(0, 1),
                            start=False, stop=False)
                    nc.tensor.matmul(
                        sps[:, nt * 512: nt * 512 + 512],
                        ident8[:],
                        shifted[:, qb * T + nt * 512: qb * T + nt * 512 + 512],
                        start=False, stop=True)
                nc.scalar.activation(
                    E_sb[:, qi * T:(qi + 1) * T], sps[:], AF.Exp,
                    accum_out=den[:, qi:qi + 1])
                nc.vector.reciprocal(rec[:, qi:qi + 1], den[:, qi:qi + 1])
                nc.vector.tensor_scalar_mul(
                    E_sb[:, qi * T:(qi + 1) * T],
                    E_sb[:, qi * T:(qi + 1) * T], rec[:, qi:qi + 1])
            # transpose E (fp16) -> ET [keys, 256], one psum bank at a time
            ET_sb = sb2.tile([128, QB * 256], F16, tag="ET", bufs=4)
            for half in range(2):
                etps = ps_m.tile([128, 4 * 256], F16, tag="misc")
                for qi in range(2):
                    for kc in range(4):
                        kca = half * 4 + kc
                        nc.tensor.transpose(
                            etps[:, kc * 256 + qi * 128: kc * 256 + qi * 128 + 128],
                            E_sb[:, qi * T + kca * 128: qi * T + kca * 128 + 128],
                            ident16[:])
                if half == 0:
                    nc.vector.tensor_copy(ET_sb[:, :1024], etps[:])
                else:
                    nc.vector.tensor_copy(ET_sb[:, 1024:], etps[:])
            # attention @ V -> oT; head parity selects the partition half
            # so the output projection can contract a full 128 (head pair)
            off = (h % 2) * 64
            otps = ps_m.tile([128, 256], F32, tag="misc")
            for kc in range(QB):
                nc.tensor.matmul(
                    otps[off:off + 64, :],
                    v_sb[:, kc * 256 + h * 64: kc * 256 + h * 64 + 64],
                    ET_sb[:, kc * 256:(kc + 1) * 256],
                    start=(kc == 0), stop=(kc == QB - 1))
            dst = oT_sb[off:off + 64,
                        (h // 2) * T + qbp * 256: (h // 2) * T + qbp * 256 + 256]
            nc.vector.tensor_copy(dst, otps[off:off + 64, :])

        def emit_outproj(t8):
            ops_ = ps_m.tile([128, 512], F32, tag="misc")
            for p in range(NP):
                nc.tensor.matmul(
                    ops_[:],
                    oT_sb[:, p * T + t8 * 128: p * T + t8 * 128 + 128],
                    wo_sb[:, p * D:(p + 1) * D],
                    start=(p == 0), stop=(p == NP - 1))
            osb = sb2.tile([128, 512], F32, tag="osb", bufs=4)
            nc.vector.tensor_copy(osb[:], ops_[:])
            nc.sync.dma_start(out_d[t8 * 128:(t8 + 1) * 128, :], osb[:])

        # prologue: pair-0 projections first (heads 0 and 1 need only those),
        # then bands for heads 0/1 woven with pair-1 and v projections.
        alloc_shifted(0)
        alloc_shifted(1)
        emit_apply(0)
        emit_qk(0, 0)
        emit_band_qb(0, 0)
        emit_band_qb(0, 1)
        emit_skew_read(0, 0)
        emit_apply(1)
        emit_band_qb(0, 2)
        emit_band_qb(0, 3)
        emit_skew_read(0, 1)
        emit_qk(0, 1)
        emit_v(0)
        emit_v(1)
        emit_qk(1, 0)
        emit_v(2)
        emit_v(3)
        emit_band_qb(0, 4)
        emit_band_qb(0, 5)
        emit_skew_read(0, 2)
        emit_qk(1, 1)
        emit_band_qb(0, 6)
        emit_band_qb(0, 7)
        emit_skew_read(0, 3)
        for part in range(4):
            emit_band_qb(1, 2 * part)
            emit_band_qb(1, 2 * part + 1)
            emit_skew_read(1, part)
            emit_v(4 + part)
        # steady state: bands of head h+2 woven ahead of attention of head h.
        # The skewed re-read for head h+1 is issued during head h: its bounce
        # writes completed a head earlier, and its SBUF buffer (shared with
        # shifted[h-1], bufs=2) has no readers left by then. Stage B (attn@V)
        # trails stage A by one qbp so the XBAR transpose latency hides
        # behind the next group's score matmuls; the output projection is
        # woven into the last head.
        for h in range(NH):
            hn = h + 2
            for qbp in range(QB // 2):
                if hn < NH:
                    if qbp == 0:
                        alloc_shifted(hn)
                    emit_band_qb(hn, 2 * qbp)
                    emit_band_qb(hn, 2 * qbp + 1)
                if h >= 1 and h + 1 < NH:
                    emit_skew_read(h + 1, qbp)
                emit_attn_qbp(h, qbp)
                if h == NH - 1 and qbp >= 1:
                    emit_outproj(2 * (qbp - 1))
                    emit_outproj(2 * (qbp - 1) + 1)
        for t8 in (6, 7):
            emit_outproj(t8)

    nc.compile()
    return nc


_PROGRAM_CACHE: dict = {}


def _get_program() -> bass.Bass:
    if "nc" not in _PROGRAM_CACHE:
        _PROGRAM_CACHE["nc"] = _build_program()
    return _PROGRAM_CACHE["nc"]


def _prepare_in_maps(x, pos, content_bias, pos_bias, gamma, beta,
                     Wq, bq, Wk, bk, Wv, bv, Wp, Wo, bo):
    x = np.asarray(x, np.float32)
    pos = np.asarray(pos, np.float32)
    gamma = np.asarray(gamma, np.float32)
    beta = np.asarray(beta, np.float32)

    # gamma folding: y = yln*gamma + beta  =>  y@W = yln@(gamma*W) + beta@W
    def fold(W):
        W = np.asarray(W, np.float32)
        return W * gamma[:, None, None], np.einsum("d,dhk->hk", beta, W)

    Wq_f, bq_f = fold(Wq)
    Wk_f, bk_f = fold(Wk)
    Wv_f, bv_f = fold(Wv)
    Wp = np.asarray(Wp, np.float32)
    Wo = np.asarray(Wo, np.float32)

    in_maps = []
    for core in range(8):
        b = core // 2
        g = core % 2
        hs = slice(4 * g, 4 * g + 4)
        # 0.125 = 1/sqrt(DK) is applied on-device via ACT scale; biases are
        # added after that scale, so pre-scale them here.
        qcb = 0.125 * (np.asarray(bq) + np.asarray(content_bias) + bq_f)[hs]
        qpb = 0.125 * (np.asarray(bq) + np.asarray(pos_bias) + bq_f)[hs]
        kb = (np.asarray(bk) + bk_f)[hs]
        vb = (np.asarray(bv) + bv_f)[hs]
        bf16 = mybir.dt.np(BF16)
        in_maps.append({
            "xT": np.ascontiguousarray(x[b].T),
            "posT": np.ascontiguousarray(pos[b].T.astype(mybir.dt.np(F8))),
            "wq": np.ascontiguousarray(
                Wq_f[:, hs, :].reshape(D, NH * DK).astype(bf16)),
            "wk": np.ascontiguousarray(
                Wk_f[:, hs, :].reshape(D, NH * DK).astype(bf16)),
            "wv": np.ascontiguousarray(
                Wv_f[:, hs, :].reshape(D, NH * DK).astype(bf16)),
            "wp": np.ascontiguousarray(
                Wp[:, hs, :].reshape(D, NH * DK).astype(np.float16)),
            "wo": np.ascontiguousarray(
                np.asarray(Wo)[hs].reshape(NP, 2 * DK, D)
                .transpose(1, 0, 2).reshape(2 * DK, NP * D).astype(bf16)),
            "qc_bias": np.ascontiguousarray(qcb.reshape(2, 128).T),
            "qp_bias": np.ascontiguousarray(qpb.reshape(2, 128).T),
            "k_bias": np.ascontiguousarray(kb.reshape(2, 128).T),
            "v_bias": np.ascontiguousarray(vb.reshape(NH * DK)),
        })

    return in_maps


def _combine(x, bo, results):
    parts = [r["out_partial"] for r in results]
    out = np.asarray(x, np.float32) + np.asarray(bo, np.float32)[None, None, :]
    for b in range(B):
        out[b] += parts[2 * b] + parts[2 * b + 1]
    return out.astype(np.float32)


def kernel(x, pos, content_bias, pos_bias, gamma, beta,
           Wq, bq, Wk, bk, Wv, bv, Wp, Wo, bo) -> np.ndarray:
    in_maps = _prepare_in_maps(x, pos, content_bias, pos_bias, gamma, beta,
                               Wq, bq, Wk, bk, Wv, bv, Wp, Wo, bo)
    nc = _get_program()
    res = run_bass_kernel_spmd(nc, in_maps, core_ids=list(range(8)))
    return _combine(x, bo, res.results)
